# revision 7
# baseline (speedup 1.0000x reference)
"""Trainium2 Bass kernel for batched attention (nn_Attention_5068061409491).

Reference computation (per batch b):
    Q = x @ Wq + bq ; K = x @ Wk + bk ; V = x @ Wv + bv          [N, 512]
    S = Q @ K^T                                                   [N, N]
    out = (softmax(S, -1) * sqrt(DK)) @ V                         [N, 512]

Sharding: pure data-parallel — B == n_cores == 8, so core i computes batch
element i independently.  No collectives.

Per-core algorithm (layouts chosen so TensorE contracts over partitions):
  - x^T [D, N] built on-chip via TensorE transposes of natural x tiles.
  - Q^T, K^T [DK, N] = Wq/Wk (stationary) @ x^T (moving); bias added via
    DVE per-partition tensor_scalar during the PSUM->SBUF copy.
  - V [N, DV] = x^T (stationary) @ Wv (moving); bias folded into the final
    epilogue (rows of normalized softmax sum to 1).
  - S^T tiles [128 kv, 512 q] = K^T (stationary) @ Q^T (moving), accumulated
    over the 4 contraction chunks in PSUM.
  - Max-free stable softmax: P = exp(S - C) with fixed C=100 (score std is
    sqrt(512)≈22.6 so shifted scores stay in [-300, 30]: no overflow, and
    row maxima never underflow).  Row sums come from F=2 matmuls against a
    ones vector, reusing the stationary P^T tile.
  - O = P^T.T @ V accumulated over kv tiles in PSUM; epilogue scales by
    sqrt(DK)/rowsum (per-partition scalar) and adds sqrt(DK)*bv.

Matmuls run as float32r (TF32) by default: full bf16-rate on the PE array
(1 cycle/row at free-dim >= 256) with 10-bit mantissas.  All fp32r matmul
operands must be produced by an op that rounds to fp32r, hence the explicit
staging copies for DMA-fed weights.
"""

import os
import sys

import numpy as np

if "/opt/trn_rl_repo" not in sys.path:
    sys.path.insert(0, "/opt/trn_rl_repo")

import concourse.bass as bass  # noqa: E402
import concourse.tile as tile  # noqa: E402
from concourse import bacc, mybir  # noqa: E402
from concourse.bass import ds, ts  # noqa: E402
from concourse.bass_utils import run_bass_kernel_spmd  # noqa: E402
from concourse.masks import make_identity  # noqa: E402

B, N, D = 8, 2048, 512
DK = DV = 512
P = 128
NT = N // P  # 16 row tiles
DC = D // P  # 4 contraction chunks
FCH = 512  # moving free-dim chunk
NCH = N // FCH  # 4 query chunks
NB = FCH // P  # 4 output row blocks per query chunk
C_SOFT = 100.0  # softmax shift constant
SQRT_DK = float(np.sqrt(DK))

F32 = mybir.dt.float32
F32R = mybir.dt.float32r
BF16 = mybir.dt.bfloat16

_DT = {"f32": F32, "f32r": F32R, "bf16": BF16}

# dtype knobs (env-overridable for experiments; defaults = shipped config)
LOGIT_DT = _DT[os.environ.get("ATT_LOGIT_DT", "f32r")]  # QT/KT & S^T matmul
PV_DT = _DT[os.environ.get("ATT_PV_DT", "f32r")]  # pt/V/ones & PV matmul


def build():
    nc = bacc.Bacc(
        "TRN2", target_bir_lowering=False, debug=False, num_devices=8
    )

    x_ext = nc.dram_tensor("x", [N, D], F32, kind="ExternalInput").ap()
    wq_ext = nc.dram_tensor("Wq", [D, DK], F32, kind="ExternalInput").ap()
    wk_ext = nc.dram_tensor("Wk", [D, DK], F32, kind="ExternalInput").ap()
    wv_ext = nc.dram_tensor("Wv", [D, DV], F32, kind="ExternalInput").ap()
    # host-prepped biases: bqs/bks striped [P, DK//P]; bvs = sqrt(DK)*bv
    # broadcast to [P, DV]
    bqs_ext = nc.dram_tensor("bqs", [P, DK // P], F32, kind="ExternalInput").ap()
    bks_ext = nc.dram_tensor("bks", [P, DK // P], F32, kind="ExternalInput").ap()
    bvs_ext = nc.dram_tensor("bvs", [P, DV], F32, kind="ExternalInput").ap()
    out_ext = nc.dram_tensor("out", [N, DV], F32, kind="ExternalOutput").ap()

    with tile.TileContext(nc) as tc:
        with tc.tile_pool(name="persist", bufs=1) as persist:
            xT = persist.tile([P, DC, N], F32R, name="xT")
            QT = persist.tile([P, DC, N], LOGIT_DT, name="QT")
            KT = persist.tile([P, DC, N], LOGIT_DT, name="KT")
            Vsb = persist.tile([P, NT, DV], PV_DT, name="Vsb")
            wq_sb = persist.tile([P, DC, DK], F32R, name="wq_sb")
            wk_sb = persist.tile([P, DC, DK], F32R, name="wk_sb")
            wv_sb = persist.tile([P, DC, DV], F32R, name="wv_sb")
            bqs_sb = persist.tile([P, DC], F32, name="bqs_sb")
            bks_sb = persist.tile([P, DC], F32, name="bks_sb")
            bvs_sb = persist.tile([P, DV], F32, name="bvs_sb")
            ones_f32 = persist.tile([P, 2], F32, name="ones_f32")
            ones_sb = persist.tile([P, 2], PV_DT, name="ones_sb")
            negc_sb = persist.tile([P, 1], F32, name="negc_sb")
            ident = persist.tile([P, P], F32, name="ident")

            nc.sync.dma_start(bqs_sb[:], bqs_ext)
            nc.sync.dma_start(bks_sb[:], bks_ext)
            nc.sync.dma_start(bvs_sb[:], bvs_ext)
            nc.vector.memset(ones_f32[:], 1.0)
            nc.vector.tensor_copy(out=ones_sb[:], in_=ones_f32[:])
            nc.vector.memset(negc_sb[:], -C_SOFT)
            make_identity(nc, ident[:])

            # weights: DMA to staging, then round-copy to fp32r tiles
            with tc.tile_pool(name="wstage", bufs=2) as wstage:
                for w_sb, w_ext in (
                    (wq_sb, wq_ext),
                    (wk_sb, wk_ext),
                    (wv_sb, wv_ext),
                ):
                    wst = wstage.tile(
                        [P, DC, DK], F32, tag="wst", name=f"wst_{w_sb.name}"
                    )
                    nc.sync.dma_start(
                        wst[:], w_ext.rearrange("(po pi) k -> pi po k", pi=P)
                    )
                    nc.vector.tensor_copy(out=w_sb[:], in_=wst[:])

            # ---- phase 1a: x^T via TensorE transposes --------------------
            with tc.tile_pool(name="xstage", bufs=3) as xstage, tc.tile_pool(
                name="tpsum", bufs=4, space="PSUM"
            ) as tpsum:
                for t in range(NT):
                    x_nat = xstage.tile([P, D], F32, tag="xnat", name=f"xnat{t}")
                    nc.sync.dma_start(x_nat[:], x_ext[ts(t, P), :])
                    for dc in range(DC):
                        ps = tpsum.tile([P, P], F32, tag="tps", name=f"tps{t}_{dc}")
                        nc.tensor.transpose(ps[:], x_nat[:, ts(dc, P)], ident[:])
                        nc.vector.tensor_copy(out=xT[:, dc, ts(t, P)], in_=ps[:])

            # ---- phase 1b: Q^T, K^T, V projections -----------------------
            with tc.tile_pool(name="qkvpsum", bufs=4, space="PSUM") as qpsum:
                for dst, w_sb, b_sb in ((QT, wq_sb, bqs_sb), (KT, wk_sb, bks_sb)):
                    for po in range(DC):  # output dk block
                        for nch in range(NCH):
                            ps = qpsum.tile(
                                [P, FCH], F32, tag="qkv",
                                name=f"qkvps_{dst.name}_{po}_{nch}",
                            )
                            for dc in range(DC):
                                nc.tensor.matmul(
                                    ps[:],
                                    lhsT=w_sb[:, dc, ts(po, P)],
                                    rhs=xT[:, dc, ts(nch, FCH)],
                                    start=(dc == 0),
                                    stop=(dc == DC - 1),
                                )
                            nc.vector.tensor_scalar_add(
                                dst[:, po, ts(nch, FCH)],
                                ps[:],
                                b_sb[:, po : po + 1],
                            )
                for mt in range(NT):
                    ps = qpsum.tile([P, DV], F32, tag="qkv", name=f"vps{mt}")
                    for dc in range(DC):
                        nc.tensor.matmul(
                            ps[:],
                            lhsT=xT[:, dc, ts(mt, P)],
                            rhs=wv_sb[:, dc, :],
                            start=(dc == 0),
                            stop=(dc == DC - 1),
                        )
                    nc.vector.tensor_copy(out=Vsb[:, mt, :], in_=ps[:])

            # ---- phase 2: attention --------------------------------------
            with tc.tile_pool(name="stpsum", bufs=3, space="PSUM") as stp, \
                    tc.tile_pool(name="opsum", bufs=4, space="PSUM") as op, \
                    tc.tile_pool(name="rpsum", bufs=1, space="PSUM") as rp, \
                    tc.tile_pool(name="ptpool", bufs=3) as ptpool, \
                    tc.tile_pool(name="epi", bufs=8) as epi:
                for nch in range(NCH):
                    o_ps = [
                        op.tile([P, DV], F32, tag="o", name=f"ops{nch}_{nb}")
                        for nb in range(NB)
                    ]
                    r_ps = rp.tile([P, 2 * NB], F32, tag="r", name=f"rps{nch}")
                    for mt in range(NT):
                        st = stp.tile([P, FCH], F32, tag="st", name=f"st{nch}_{mt}")
                        for dc in range(DC):
                            nc.tensor.matmul(
                                st[:],
                                lhsT=KT[:, dc, ts(mt, P)],
                                rhs=QT[:, dc, ts(nch, FCH)],
                                start=(dc == 0),
                                stop=(dc == DC - 1),
                            )
                        pt = ptpool.tile(
                            [P, FCH], PV_DT, tag="pt", name=f"pt{nch}_{mt}"
                        )
                        nc.scalar.activation(
                            out=pt[:],
                            in_=st[:],
                            func=mybir.ActivationFunctionType.Exp,
                            bias=negc_sb[:],
                            scale=1.0,
                        )
                        for nb in range(NB):
                            nc.tensor.matmul(
                                o_ps[nb][:],
                                lhsT=pt[:, ts(nb, P)],
                                rhs=Vsb[:, mt, :],
                                start=(mt == 0),
                                stop=(mt == NT - 1),
                            )
                            # NB: start=True clears the ENTIRE psum bank
                            # (first_mm semantics), and all 4 nb rowsum
                            # groups share one bank.  Only the very first
                            # matmul clears; the other groups' first writes
                            # land on has_written=0 elements and overwrite.
                            nc.tensor.matmul(
                                r_ps[:, 2 * nb : 2 * nb + 2],
                                lhsT=pt[:, ts(nb, P)],
                                rhs=ones_sb[:],
                                start=(mt == 0 and nb == 0),
                                stop=(mt == NT - 1),
                                skip_group_check=True,
                            )
                    # epilogue: out = sqrt(DK)/r * O + sqrt(DK)*bv
                    rsc = epi.tile([P, 2 * NB], F32, tag="rsc", name=f"rsc{nch}")
                    nc.scalar.mul(rsc[:], r_ps[:], 1.0 / SQRT_DK)
                    rinv = epi.tile([P, 2 * NB], F32, tag="rinv", name=f"rinv{nch}")
                    nc.vector.reciprocal(rinv[:], rsc[:])
                    for nb in range(NB):
                        o_sb = epi.tile([P, DV], F32, tag="osb", name=f"osb{nch}_{nb}")
                        nc.vector.tensor_scalar_mul(
                            o_sb[:], o_ps[nb][:], rinv[:, 2 * nb : 2 * nb + 1]
                        )
                        nc.vector.tensor_add(o_sb[:], o_sb[:], bvs_sb[:])
                        nc.sync.dma_start(
                            out_ext[ds(nch * FCH + nb * P, P), :], o_sb[:]
                        )

    nc.compile()
    return nc


_NC_CACHE = None


def _get_nc():
    global _NC_CACHE
    if _NC_CACHE is None:
        _NC_CACHE = build()
    return _NC_CACHE


def _prep_in_maps(x, Wq, bq, Wk, bk, Wv, bv):
    x = np.asarray(x, dtype=np.float32)
    Wq = np.ascontiguousarray(np.asarray(Wq, dtype=np.float32))
    Wk = np.ascontiguousarray(np.asarray(Wk, dtype=np.float32))
    Wv = np.ascontiguousarray(np.asarray(Wv, dtype=np.float32))
    bqs = np.ascontiguousarray(np.asarray(bq, np.float32).reshape(DC, P).T)
    bks = np.ascontiguousarray(np.asarray(bk, np.float32).reshape(DC, P).T)
    bvs = np.ascontiguousarray(
        np.broadcast_to(np.asarray(bv, np.float32) * SQRT_DK, (P, DV))
    )
    return [
        {
            "x": np.ascontiguousarray(x[i]),
            "Wq": Wq,
            "Wk": Wk,
            "Wv": Wv,
            "bqs": bqs,
            "bks": bks,
            "bvs": bvs,
        }
        for i in range(B)
    ]


def kernel(x, Wq, bq, Wk, bk, Wv, bv):
    nc = _get_nc()
    in_maps = _prep_in_maps(x, Wq, bq, Wk, bk, Wv, bv)
    res = run_bass_kernel_spmd(nc, in_maps, core_ids=list(range(B)))
    return np.stack([r["out"] for r in res.results], axis=0)


# revision 8
# speedup vs baseline: 2.9212x; 2.9212x over previous
"""Trainium2 Bass kernel for batched attention (nn_Attention_5068061409491).

Reference computation (per batch b):
    Q = x @ Wq + bq ; K = x @ Wk + bk ; V = x @ Wv + bv          [N, 512]
    S = Q @ K^T                                                   [N, N]
    out = (softmax(S, -1) * sqrt(DK)) @ V                         [N, 512]

Sharding: pure data-parallel — B == n_cores == 8, so core i computes batch
element i independently.  No collectives.

Per-core algorithm (layouts chosen so TensorE contracts over partitions):
  - x^T [D, N] built on-chip via TensorE transposes of natural x tiles.
  - Q^T, K^T [DK, N] = Wq/Wk (stationary) @ x^T (moving); bias added via
    DVE per-partition tensor_scalar during the PSUM->SBUF copy.
  - V [N, DV] = x^T (stationary) @ Wv (moving); bias folded into the final
    epilogue (rows of normalized softmax sum to 1).
  - S^T tiles [128 kv, 512 q] = K^T (stationary) @ Q^T (moving), accumulated
    over the 4 contraction chunks in PSUM.
  - Max-free stable softmax: P = exp(S - C) with fixed C=100 (score std is
    sqrt(512)≈22.6 so shifted scores stay in [-300, 30]: no overflow, and
    row maxima never underflow).  Row sums come from F=2 matmuls against a
    ones vector, reusing the stationary P^T tile.
  - O = P^T.T @ V accumulated over kv tiles in PSUM; epilogue scales by
    sqrt(DK)/rowsum (per-partition scalar) and adds sqrt(DK)*bv.

Matmuls run as float32r (TF32) by default: full bf16-rate on the PE array
(1 cycle/row at free-dim >= 256) with 10-bit mantissas.  All fp32r matmul
operands must be produced by an op that rounds to fp32r, hence the explicit
staging copies for DMA-fed weights.
"""

import os
import sys

import numpy as np

if "/opt/trn_rl_repo" not in sys.path:
    sys.path.insert(0, "/opt/trn_rl_repo")

import concourse.bass as bass  # noqa: E402
import concourse.tile as tile  # noqa: E402
from concourse import bacc, mybir  # noqa: E402
from concourse.bass import ds, ts  # noqa: E402
from concourse.bass_utils import run_bass_kernel_spmd  # noqa: E402
from concourse.masks import make_identity  # noqa: E402

B, N, D = 8, 2048, 512
DK = DV = 512
P = 128
NT = N // P  # 16 row tiles
DC = D // P  # 4 contraction chunks
FCH = 512  # moving free-dim chunk
NCH = N // FCH  # 4 query chunks
NB = FCH // P  # 4 output row blocks per query chunk
C_SOFT = 100.0  # softmax shift constant
SQRT_DK = float(np.sqrt(DK))

F32 = mybir.dt.float32
F32R = mybir.dt.float32r
BF16 = mybir.dt.bfloat16

_DT = {"f32": F32, "f32r": F32R, "bf16": BF16}

# dtype knobs (env-overridable for experiments; defaults = shipped config)
LOGIT_DT = _DT[os.environ.get("ATT_LOGIT_DT", "f32r")]  # QT/KT & S^T matmul
PV_DT = _DT[os.environ.get("ATT_PV_DT", "f32r")]  # pt/V/ones & PV matmul


def build(n_iters=1):
    nc = bacc.Bacc(
        "TRN2", target_bir_lowering=False, debug=False, num_devices=8
    )

    x_ext = nc.dram_tensor("x", [N, D], F32, kind="ExternalInput").ap()
    wq_ext = nc.dram_tensor("Wq", [D, DK], F32, kind="ExternalInput").ap()
    wk_ext = nc.dram_tensor("Wk", [D, DK], F32, kind="ExternalInput").ap()
    wv_ext = nc.dram_tensor("Wv", [D, DV], F32, kind="ExternalInput").ap()
    # host-prepped biases: bqs/bks striped [P, DK//P]; bvs = sqrt(DK)*bv
    # broadcast to [P, DV]
    bqs_ext = nc.dram_tensor("bqs", [P, DK // P], F32, kind="ExternalInput").ap()
    bks_ext = nc.dram_tensor("bks", [P, DK // P], F32, kind="ExternalInput").ap()
    bvs_ext = nc.dram_tensor("bvs", [P, DV], F32, kind="ExternalInput").ap()
    out_ext = nc.dram_tensor("out", [N, DV], F32, kind="ExternalOutput").ap()

    with tile.TileContext(nc) as tc:
      for _it in range(n_iters):
        with tc.tile_pool(name=f"persist{_it}", bufs=1) as persist:
            xT = persist.tile([P, DC, N], F32R, name="xT")
            QT = persist.tile([P, DC, N], LOGIT_DT, name="QT")
            KT = persist.tile([P, DC, N], LOGIT_DT, name="KT")
            Vsb = persist.tile([P, NT, DV], PV_DT, name="Vsb")
            wq_sb = persist.tile([P, DC, DK], F32R, name="wq_sb")
            wk_sb = persist.tile([P, DC, DK], F32R, name="wk_sb")
            wv_sb = persist.tile([P, DC, DV], F32R, name="wv_sb")
            bqs_sb = persist.tile([P, DC], F32, name="bqs_sb")
            bks_sb = persist.tile([P, DC], F32, name="bks_sb")
            bvs_sb = persist.tile([P, DV], F32, name="bvs_sb")
            ones_f32 = persist.tile([P, 2], F32, name="ones_f32")
            ones_sb = persist.tile([P, 2], PV_DT, name="ones_sb")
            negc_sb = persist.tile([P, 1], F32, name="negc_sb")
            ident = persist.tile([P, P], F32, name="ident")

            nc.sync.dma_start(bqs_sb[:], bqs_ext)
            nc.sync.dma_start(bks_sb[:], bks_ext)
            nc.sync.dma_start(bvs_sb[:], bvs_ext)
            nc.vector.memset(ones_f32[:], 1.0)
            nc.vector.tensor_copy(out=ones_sb[:], in_=ones_f32[:])
            nc.vector.memset(negc_sb[:], -C_SOFT)
            make_identity(nc, ident[:])

            # weights: DMA to staging, then round-copy to fp32r tiles
            with tc.tile_pool(name="wstage", bufs=2) as wstage:
                for w_sb, w_ext in (
                    (wq_sb, wq_ext),
                    (wk_sb, wk_ext),
                    (wv_sb, wv_ext),
                ):
                    wst = wstage.tile(
                        [P, DC, DK], F32, tag="wst", name=f"wst_{w_sb.name}"
                    )
                    nc.sync.dma_start(
                        wst[:], w_ext.rearrange("(po pi) k -> pi po k", pi=P)
                    )
                    nc.vector.tensor_copy(out=w_sb[:], in_=wst[:])

            # ---- phase 1a: x^T via TensorE transposes --------------------
            with tc.tile_pool(name="xstage", bufs=3) as xstage, tc.tile_pool(
                name="tpsum", bufs=4, space="PSUM"
            ) as tpsum:
                for t in range(NT):
                    x_nat = xstage.tile([P, D], F32, tag="xnat", name=f"xnat{t}")
                    nc.sync.dma_start(x_nat[:], x_ext[ts(t, P), :])
                    for dc in range(DC):
                        ps = tpsum.tile([P, P], F32, tag="tps", name=f"tps{t}_{dc}")
                        nc.tensor.transpose(ps[:], x_nat[:, ts(dc, P)], ident[:])
                        nc.vector.tensor_copy(out=xT[:, dc, ts(t, P)], in_=ps[:])

            # ---- phase 1b: Q^T, K^T, V projections -----------------------
            with tc.tile_pool(name="qkvpsum", bufs=4, space="PSUM") as qpsum:
                for dst, w_sb, b_sb in ((QT, wq_sb, bqs_sb), (KT, wk_sb, bks_sb)):
                    for po in range(DC):  # output dk block
                        for nch in range(NCH):
                            ps = qpsum.tile(
                                [P, FCH], F32, tag="qkv",
                                name=f"qkvps_{dst.name}_{po}_{nch}",
                            )
                            for dc in range(DC):
                                nc.tensor.matmul(
                                    ps[:],
                                    lhsT=w_sb[:, dc, ts(po, P)],
                                    rhs=xT[:, dc, ts(nch, FCH)],
                                    start=(dc == 0),
                                    stop=(dc == DC - 1),
                                )
                            nc.vector.tensor_scalar_add(
                                dst[:, po, ts(nch, FCH)],
                                ps[:],
                                b_sb[:, po : po + 1],
                            )
                for mt in range(NT):
                    ps = qpsum.tile([P, DV], F32, tag="qkv", name=f"vps{mt}")
                    for dc in range(DC):
                        nc.tensor.matmul(
                            ps[:],
                            lhsT=xT[:, dc, ts(mt, P)],
                            rhs=wv_sb[:, dc, :],
                            start=(dc == 0),
                            stop=(dc == DC - 1),
                        )
                    nc.vector.tensor_copy(out=Vsb[:, mt, :], in_=ps[:])

            # ---- phase 2: attention --------------------------------------
            with tc.tile_pool(name="stpsum", bufs=3, space="PSUM") as stp, \
                    tc.tile_pool(name="opsum", bufs=4, space="PSUM") as op, \
                    tc.tile_pool(name="rpsum", bufs=1, space="PSUM") as rp, \
                    tc.tile_pool(name="ptpool", bufs=3) as ptpool, \
                    tc.tile_pool(name="epi", bufs=8) as epi:
                for nch in range(NCH):
                    o_ps = [
                        op.tile([P, DV], F32, tag="o", name=f"ops{nch}_{nb}")
                        for nb in range(NB)
                    ]
                    r_ps = rp.tile([P, 2 * NB], F32, tag="r", name=f"rps{nch}")
                    for mt in range(NT):
                        st = stp.tile([P, FCH], F32, tag="st", name=f"st{nch}_{mt}")
                        for dc in range(DC):
                            nc.tensor.matmul(
                                st[:],
                                lhsT=KT[:, dc, ts(mt, P)],
                                rhs=QT[:, dc, ts(nch, FCH)],
                                start=(dc == 0),
                                stop=(dc == DC - 1),
                            )
                        pt = ptpool.tile(
                            [P, FCH], PV_DT, tag="pt", name=f"pt{nch}_{mt}"
                        )
                        nc.scalar.activation(
                            out=pt[:],
                            in_=st[:],
                            func=mybir.ActivationFunctionType.Exp,
                            bias=negc_sb[:],
                            scale=1.0,
                        )
                        for nb in range(NB):
                            nc.tensor.matmul(
                                o_ps[nb][:],
                                lhsT=pt[:, ts(nb, P)],
                                rhs=Vsb[:, mt, :],
                                start=(mt == 0),
                                stop=(mt == NT - 1),
                            )
                            # NB: start=True clears the ENTIRE psum bank
                            # (first_mm semantics), and all 4 nb rowsum
                            # groups share one bank.  Only the very first
                            # matmul clears; the other groups' first writes
                            # land on has_written=0 elements and overwrite.
                            nc.tensor.matmul(
                                r_ps[:, 2 * nb : 2 * nb + 2],
                                lhsT=pt[:, ts(nb, P)],
                                rhs=ones_sb[:],
                                start=(mt == 0 and nb == 0),
                                stop=(mt == NT - 1),
                                skip_group_check=True,
                            )
                    # epilogue: out = sqrt(DK)/r * O + sqrt(DK)*bv
                    rsc = epi.tile([P, 2 * NB], F32, tag="rsc", name=f"rsc{nch}")
                    nc.scalar.mul(rsc[:], r_ps[:], 1.0 / SQRT_DK)
                    rinv = epi.tile([P, 2 * NB], F32, tag="rinv", name=f"rinv{nch}")
                    nc.vector.reciprocal(rinv[:], rsc[:])
                    for nb in range(NB):
                        o_sb = epi.tile([P, DV], F32, tag="osb", name=f"osb{nch}_{nb}")
                        nc.vector.tensor_scalar_mul(
                            o_sb[:], o_ps[nb][:], rinv[:, 2 * nb : 2 * nb + 1]
                        )
                        nc.vector.tensor_add(o_sb[:], o_sb[:], bvs_sb[:])
                        nc.sync.dma_start(
                            out_ext[ds(nch * FCH + nb * P, P), :], o_sb[:]
                        )

    nc.compile()
    return nc


_NC_CACHE = {}


def _get_nc(n_iters=1):
    if n_iters not in _NC_CACHE:
        _NC_CACHE[n_iters] = build(n_iters)
    return _NC_CACHE[n_iters]


def _prep_in_maps(x, Wq, bq, Wk, bk, Wv, bv):
    x = np.asarray(x, dtype=np.float32)
    Wq = np.ascontiguousarray(np.asarray(Wq, dtype=np.float32))
    Wk = np.ascontiguousarray(np.asarray(Wk, dtype=np.float32))
    Wv = np.ascontiguousarray(np.asarray(Wv, dtype=np.float32))
    bqs = np.ascontiguousarray(np.asarray(bq, np.float32).reshape(DC, P).T)
    bks = np.ascontiguousarray(np.asarray(bk, np.float32).reshape(DC, P).T)
    bvs = np.ascontiguousarray(
        np.broadcast_to(np.asarray(bv, np.float32) * SQRT_DK, (P, DV))
    )
    return [
        {
            "x": np.ascontiguousarray(x[i]),
            "Wq": Wq,
            "Wk": Wk,
            "Wv": Wv,
            "bqs": bqs,
            "bks": bks,
            "bvs": bvs,
        }
        for i in range(B)
    ]


def kernel(x, Wq, bq, Wk, bk, Wv, bv):
    nc = _get_nc()
    in_maps = _prep_in_maps(x, Wq, bq, Wk, bk, Wv, bv)
    res = run_bass_kernel_spmd(nc, in_maps, core_ids=list(range(B)))
    return np.stack([r["out"] for r in res.results], axis=0)
